# revision 42
# baseline (speedup 1.0000x reference)
"""LowRankSparse2to4Linear Trainium2 kernel.

out = (x16 @ A16) -> fp16 -> (@ B16^T) + bias, where A16/B16 are the 2:4
soft-thresholded (along rank), scaled, fp16-cast low-rank factors.

Strategy (8 NeuronCores, data-parallel over tokens, NO collectives):
  - tokens (8192) sharded 1024/core; every core receives the FULL weights
    and redundantly preprocesses them on-chip.
  - 2:4 soft-threshold in NATURAL rank layout, processed in CHUNK PAIRS
    (two 128-row chunks side by side in one (128, 2048) tile) to halve
    per-instruction overhead: one contiguous ACT Abs per pair, packed
    pair-min/max on DVE using the (0,2)/(1,3) pairing (the
    2nd-smallest-of-4 tournament is valid for any disjoint pairing),
    E/F/t on DVE, fused custom SOFT_SHRINK reading raw f32.
  - GEMM1 computes x_proj^T = A_sp^T @ x^T (rank-major) with a single
    8-bank PSUM sweep per token-half (all 8 rank chunks at once) so
    LDWEIGHTS stays hidden under 512-row matmuls.
  - x is loaded directly as fp16 via casting SWDGE DMAs (gpsimd software
    DGE converts dtypes in flight) -- no separate cast ops, no f32 x
    staging.  x^T comes from PE transposes (PSUM) + ACT copies.
  - Engine balance: ACT does Abs + xT/x_proj/output copies and half the
    wbt copies; DVE does the tournament + shrink + the other half of the
    wbt copies.  Program order interleaves producers with the PE stream
    (x blocks lead by two, B^T blocks by one output block) so no engine
    queue head-blocks the PE.
  - B preprocessing is split: half during the th=1 GEMM1 window, half
    just-in-time inside the GEMM2 loop.
"""

import os
import sys
import numpy as np

sys.path.insert(0, "/opt/trn_rl_repo")

N_CORES = 8
IN_F, OUT_F, RANK = 4096, 4096, 1024
T_FULL = 8192             # 4 * 2048 tokens
TPC = T_FULL // N_CORES   # 1024 tokens per core

_BUILD_CACHE = {}


_DVE_OPS = {}


def _register_custom_dve_ops():
    """Register the fused soft-shrink DVE op (runtime extension of
    concourse.dve_ops):  SOFT_SHRINK: out = in0 - clamp(in0, -in1, in1)."""
    if _DVE_OPS:
        return _DVE_OPS
    import numpy as _np
    from concourse import dve_ops
    from concourse.dve_spec import (Spec, Src0, Src1, Zero, minn, maxx,
                                    select, lower, _has_src1)
    from concourse.dve_uop import DveOpSpec

    def make_op(name, body, ref):
        existing = {op.name: op for op in dve_ops.OPS}
        if name in existing:
            return existing[name]
        spec = Spec(body=body, reference=ref)
        row = dve_ops._CUSTOM_DVE_ROW_BASE + len(dve_ops.OPS)
        shas = {}
        for ver in ("v3", "v4"):
            try:
                tmp = DveOpSpec(name=name, opcode=row, uops=lower(spec, ver=ver),
                                rd1_en=_has_src1(spec))
                shas[ver] = tmp.sha(ver)
            except Exception:
                pass
        op = dve_ops.DveOp(name, spec, subdim=False, uops_sha=shas)
        dve_ops.OPS.append(op)
        dve_ops.CUSTOM_DVE_SPECS[name] = spec
        dve_ops._SUB_OPCODE_FOR_NAME[name] = row
        return op

    _DVE_OPS["shrink"] = make_op(
        "SOFT_SHRINK_ANT",
        select(Src0 < Zero, minn(Src0 + Src1, Zero), maxx(Src0 - Src1, Zero)),
        lambda in0, in1, s0, s1, imm2: _np.where(
            in0 < 0, _np.minimum(in0 + in1, 0), _np.maximum(in0 - in1, 0)))
    return _DVE_OPS


def _build(scale_a: float, scale_b: float, bias_zero: bool):
    import concourse.bacc as bacc
    import concourse.tile as tile
    from concourse import mybir
    from concourse.masks import make_identity

    ops = _register_custom_dve_ops()

    f32 = mybir.dt.float32
    f16 = mybir.dt.float16
    Alu = mybir.AluOpType
    AF = mybir.ActivationFunctionType

    nc = bacc.Bacc("TRN2", target_bir_lowering=False, debug=False,
                   num_devices=N_CORES)

    x_sh = nc.dram_tensor("x_sh", [TPC, IN_F], f32, kind="ExternalInput")
    wa_d = nc.dram_tensor("wa_d", [IN_F, RANK], f32, kind="ExternalInput")
    wb_d = nc.dram_tensor("wb_d", [OUT_F, RANK], f32, kind="ExternalInput")
    bias_d = nc.dram_tensor("bias_d", [1, OUT_F], f32, kind="ExternalInput")
    out_d = nc.dram_tensor("out_d", [TPC, OUT_F], f32, kind="ExternalOutput")

    K_IN = IN_F // 128    # 32 contraction chunks for GEMM1
    K_RK = RANK // 128    # 8 contraction chunks for GEMM2
    N_TOK = TPC // 128    # 8 token chunks per core
    W2 = 2 * RANK         # chunk-pair width

    with tile.TileContext(nc) as tc:
        with (
            tc.tile_pool(name="singles", bufs=1) as singles,
            tc.tile_pool(name="wst", bufs=3) as p_wst,
            tc.tile_pool(name="mag", bufs=2) as p_mag,
            tc.tile_pool(name="pq", bufs=3) as p_pq,
            tc.tile_pool(name="eft", bufs=6) as p_eft,
            tc.tile_pool(name="wasp", bufs=16) as p_wasp,
            tc.tile_pool(name="wbsp", bufs=4) as p_wbsp,
            tc.tile_pool(name="x16", bufs=7) as p_x16,
            tc.tile_pool(name="xt", bufs=32) as p_xt,
            tc.tile_pool(name="xproj", bufs=16) as p_xp,
            tc.tile_pool(name="wbt", bufs=16) as p_wbt,
            tc.tile_pool(name="oev", bufs=2) as p_out,
            tc.tile_pool(name="ps", bufs=8, space="PSUM") as p_ps,
        ):
            ident = singles.tile([128, 128], f16)
            make_identity(nc, ident[:])

            def soft24_pair(src_dram, pair, scale, dst_pool, name):
                """2:4 soft-threshold TWO (128, RANK) f32 row chunks (rows
                256*pair .. 256*pair+255) into one (128, 2*RANK) fp16 tile,
                natural rank layout; chunk c occupies columns [c*RANK,
                (c+1)*RANK).

                Tournament with pairing (0,2)/(1,3) inside each rank group
                of 4 keeps every DVE access pattern packed."""
                r0 = pair * 256
                st = p_wst.tile([128, W2], f32, tag="wst",
                                name=f"wst_{name}")
                nc.sync.dma_start(
                    st[:].rearrange("p (c m) -> p c m", c=2),
                    src_dram[r0:r0 + 256, :].rearrange(
                        "(c p) m -> p c m", c=2))
                if scale != 1.0:
                    nc.scalar.mul(st[:], st[:], float(scale))
                st4 = st[:].rearrange("p (q f) -> p q f", f=4)

                M = p_mag.tile([128, W2], f16, tag="mag", name=f"M_{name}")
                nc.scalar.activation(M[:], st[:], AF.Abs)
                M4 = M[:].rearrange("p (q f) -> p q f", f=4)

                P = p_pq.tile([128, W2 // 2], f16, tag="pq",
                              name=f"P_{name}")
                Q = p_pq.tile([128, W2 // 2], f16, tag="pq",
                              name=f"Q_{name}")
                P2 = P[:].rearrange("p (q f) -> p q f", f=2)
                Q2 = Q[:].rearrange("p (q f) -> p q f", f=2)
                nc.vector.tensor_tensor(out=P2, in0=M4[:, :, 0:2],
                                        in1=M4[:, :, 2:4], op=Alu.min)
                nc.vector.tensor_tensor(out=Q2, in0=M4[:, :, 0:2],
                                        in1=M4[:, :, 2:4], op=Alu.max)

                E = p_eft.tile([128, W2 // 4], f16, tag="eft",
                               name=f"E_{name}")
                F = p_eft.tile([128, W2 // 4], f16, tag="eft",
                               name=f"F_{name}")
                t = p_eft.tile([128, W2 // 4], f16, tag="eft",
                               name=f"t_{name}")
                nc.vector.tensor_tensor(out=E[:], in0=P2[:, :, 0],
                                        in1=P2[:, :, 1], op=Alu.max)
                nc.vector.tensor_tensor(out=F[:], in0=Q2[:, :, 0],
                                        in1=Q2[:, :, 1], op=Alu.min)
                nc.vector.tensor_tensor(out=t[:], in0=E[:], in1=F[:],
                                        op=Alu.min)

                wsp = dst_pool.tile([128, W2], f16, tag="wsp",
                                    name=f"wsp_{name}")
                nc.vector._custom_dve(
                    ops["shrink"],
                    out=wsp[:].rearrange("p (q f) -> p q f", f=4),
                    in0=st4,
                    in1=t[:, :, None].to_broadcast([128, W2 // 4, 4]))
                return wsp

            def wa_slice(wa_sp, ic, c):
                """Stationary slice for GEMM1: (in-chunk ic, rank chunk c)."""
                pair, half = ic // 2, ic % 2
                o = half * RANK + c * 128
                return wa_sp[pair][:, o:o + 128]

            def emit_x_blk(th, blk, x16):
                """Load one in-feature block (4 token tiles) of token-half
                th directly as fp16 via casting SWDGE DMAs (gpsimd software
                DGE is the only DMA path that converts dtypes in flight)."""
                for tc4 in range(4):
                    tok0 = (th * 4 + tc4) * 128
                    x16t = p_x16.tile([128, 1024], f16, tag="x16",
                                      name=f"x16_{th}_{blk}_{tc4}")
                    nc.gpsimd.dma_start(
                        x16t[:], x_sh[tok0:tok0 + 128,
                                      blk * 1024:(blk + 1) * 1024])
                    x16[(blk, tc4)] = x16t

            def emit_transpose(th, ic, x16, xT):
                """PE-transpose one in-chunk of x (4 token blocks) and copy
                PSUM -> SBUF (ACT)."""
                blk, col = ic // 8, ic % 8
                pt = p_ps.tile([128, 512], f16, tag="ps",
                               name=f"pT_{th}_{ic}")
                for tc4 in range(4):
                    nc.tensor.transpose(
                        pt[:, tc4 * 128:(tc4 + 1) * 128],
                        x16[(blk, tc4)][:, col * 128:(col + 1) * 128],
                        ident[:])
                xt = p_xt.tile([128, 512], f16, tag="xt",
                               name=f"xT_{th}_{ic}")
                nc.scalar.copy(xt[:], pt[:])
                xT[ic] = xt

            def emit_gemm1_sweep(th, wa_sp, xT, xproj):
                """Single 8-bank sweep: all 8 rank chunks accumulate at
                once."""
                accs = [p_ps.tile([128, 512], f32, tag="ps",
                                  name=f"g1_{th}_{c}")
                        for c in range(8)]
                for ic in range(K_IN):
                    for c in range(8):
                        nc.tensor.matmul(
                            accs[c][:], wa_slice(wa_sp, ic, c), xT[ic][:],
                            start=(ic == 0), stop=(ic == K_IN - 1))
                for c in range(8):
                    xp = p_xp.tile([128, 512], f16, tag="xp",
                                   name=f"xp_{th}_{c}")
                    nc.scalar.copy(xp[:], accs[c][:])
                    xproj[(th, c)] = xp

            # ---------------- emission schedule ----------------
            xproj = {}
            wa_sp = [None] * 16
            wb_sp = [None] * 16

            # A preprocessing interleaved with the th0 x pipeline: per pair
            # the ACT queue gets [Abs, xt, xt] and DVE [P,Q,E,F,t,shrink]
            # -- both ahead of the PE's per-ic consumption rate.  x blocks
            # are cast-DMA'd just-in-time (one block ahead).
            x16_0 = {}
            emit_x_blk(0, 0, x16_0)
            emit_x_blk(0, 1, x16_0)
            xT0 = [None] * K_IN
            for k in range(16):
                wa_sp[k] = soft24_pair(wa_d, k, scale_a, p_wasp, f"a{k}")
                if k in (2, 6):
                    emit_x_blk(0, k // 4 + 2, x16_0)
                emit_transpose(0, 2 * k, x16_0, xT0)
                emit_transpose(0, 2 * k + 1, x16_0, xT0)

            # bias broadcast (log-doubling), only if bias nonzero
            if not bias_zero:
                bias_bc = singles.tile([128, OUT_F], f32)
                nc.sync.dma_start(bias_bc[0:1, :], bias_d[:])
                k = 1
                while k < 128:
                    nc.sync.dma_start(bias_bc[k:2 * k, :], bias_bc[0:k, :])
                    k *= 2

            emit_gemm1_sweep(0, wa_sp, xT0, xproj)

            # th1 x pipeline; first half of B preprocessing rides along
            # (DVE: B tournament, ACT: B Abs + th1 xt copies)
            x16_1 = {}
            emit_x_blk(1, 0, x16_1)
            emit_x_blk(1, 1, x16_1)
            xT1 = [None] * K_IN
            for k in range(16):
                if k < 8:
                    wb_sp[k] = soft24_pair(wb_d, k, scale_b, p_wbsp,
                                           f"b{k}")
                if k in (2, 6):
                    emit_x_blk(1, k // 4 + 2, x16_1)
                emit_transpose(1, 2 * k, x16_1, xT1)
                emit_transpose(1, 2 * k + 1, x16_1, xT1)

            emit_gemm1_sweep(1, wa_sp, xT1, xproj)

            # GEMM2 per 512-wide output block; remaining B pairs JIT with
            # a 2-block lead.
            def wb_slice(ic, rk):
                pair, half = ic // 2, ic % 2
                o = half * RANK + rk * 128
                return wb_sp[pair][:, o:o + 128]

            # B^T tiles via PE transposes, prefetched one output block
            # ahead of the consuming matmuls; PSUM->SBUF copies split
            # between ACT and DVE so neither paces the GEMM2 stream.
            wbts = {}

            def emit_wbt_block(nb):
                for rk in range(K_RK):
                    pt = p_ps.tile([128, 512], f16, tag="ps",
                                   name=f"pB_{nb}_{rk}")
                    for wc in range(4):
                        nc.tensor.transpose(
                            pt[:, wc * 128:(wc + 1) * 128],
                            wb_slice(nb * 4 + wc, rk),
                            ident[:])
                    wt = p_wbt.tile([128, 512], f16, tag="wbt",
                                    name=f"wbt_{nb}_{rk}")
                    if rk % 2 == 0:
                        nc.scalar.copy(wt[:], pt[:])
                    else:
                        nc.vector.tensor_copy(out=wt[:], in_=pt[:])
                    wbts[(nb, rk)] = wt

            emit_wbt_block(0)
            for nb in range(OUT_F // 512):
                for k in (2 * (nb + 4), 2 * (nb + 4) + 1):
                    if k < 16 and wb_sp[k] is None:
                        wb_sp[k] = soft24_pair(wb_d, k, scale_b, p_wbsp,
                                               f"b{k}")
                if nb < 7:
                    emit_wbt_block(nb + 1)
                for mt in range(N_TOK):
                    acc2 = p_ps.tile([128, 512], f32, tag="ps",
                                     name=f"g2_{nb}_{mt}")
                    th, ml = mt // 4, mt % 4
                    for kc in range(K_RK):
                        nc.tensor.matmul(
                            acc2[:],
                            xproj[(th, kc)][:, ml * 128:(ml + 1) * 128],
                            wbts[(nb, kc)][:],
                            start=(kc == 0), stop=(kc == K_RK - 1))
                    ot = p_out.tile([128, 512], f32, tag="oev",
                                    name=f"ot_{nb}_{mt}")
                    if bias_zero:
                        if mt % 2 == 0:
                            nc.scalar.copy(ot[:], acc2[:])
                        else:
                            nc.vector.tensor_copy(out=ot[:], in_=acc2[:])
                    else:
                        nc.vector.tensor_tensor(
                            out=ot[:], in0=acc2[:],
                            in1=bias_bc[:, nb * 512:(nb + 1) * 512],
                            op=Alu.add)
                    nc.sync.dma_start(
                        out_d[mt * 128:(mt + 1) * 128,
                              nb * 512:(nb + 1) * 512],
                        ot[:])

    nc.compile()
    return nc


def kernel(x, weight_A, weight_B, bias, scale_A, scale_B):
    from concourse.bass_utils import run_bass_kernel_spmd

    x = np.ascontiguousarray(np.asarray(x, dtype=np.float32))
    weight_A = np.ascontiguousarray(np.asarray(weight_A, dtype=np.float32))
    weight_B = np.ascontiguousarray(np.asarray(weight_B, dtype=np.float32))
    bias = np.ascontiguousarray(np.asarray(bias, dtype=np.float32))
    sa = float(np.asarray(scale_A))
    sb = float(np.asarray(scale_B))
    bias_zero = bool(np.all(bias == 0.0))

    lead = x.shape[:-1]
    xf = x.reshape(-1, IN_F)
    assert xf.shape == (T_FULL, IN_F)

    key = (sa, sb, bias_zero)
    if key not in _BUILD_CACHE:
        _BUILD_CACHE[key] = _build(sa, sb, bias_zero)
    nc = _BUILD_CACHE[key]

    bias_row = bias.reshape(1, OUT_F)
    in_maps = []
    for c in range(N_CORES):
        in_maps.append({
            "x_sh": xf[c * TPC:(c + 1) * TPC],
            "wa_d": weight_A,
            "wb_d": weight_B,
            "bias_d": bias_row,
        })

    trace = os.environ.get("BASS_KERNEL_TRACE", "0") == "1"
    kwargs = {}
    if trace:
        _install_ntff_hook()
        kwargs["trace"] = True
        tmpdir = os.environ.get("BASS_KERNEL_TRACE_DIR")
        if tmpdir:
            os.makedirs(tmpdir, exist_ok=True)
            kwargs["tmpdir"] = tmpdir

    res = run_bass_kernel_spmd(nc, in_maps, core_ids=list(range(N_CORES)),
                               **kwargs)
    if trace:
        kernel.last_exec_time_ns = res.exec_time_ns

    out = np.empty((T_FULL, OUT_F), dtype=np.float32)
    for c in range(N_CORES):
        out[c * TPC:(c + 1) * TPC] = res.results[c]["out_d"]
    return out.reshape(*lead, OUT_F)


def _install_ntff_hook():
    """Provide antenv.axon_hooks (missing in this image) so trace=True works."""
    import types
    if "antenv.axon_hooks" in sys.modules:
        return
    try:
        from trn_agent_boot.trn_boot import _ntff_profile_via_ctypes
        hook = _ntff_profile_via_ctypes("/opt/axon/libaxon_pjrt.so")
    except Exception:
        hook = None
    mod = types.ModuleType("antenv.axon_hooks")
    mod.get_axon_ntff_profile_hook = lambda: hook
    mod.set_axon_ntff_profile_hook = lambda h: None
    import antenv  # noqa: F401
    sys.modules["antenv.axon_hooks"] = mod
